# revision 4
# baseline (speedup 1.0000x reference)
"""BoxFilter (9x9 unnormalized box sum, zero-padded) on 8 trn2 cores.

Minimal-instruction grouped design for an execution environment where each
instruction has a large fixed cost and big 2D DVE ops amortize best:

Per group of G channels (bf16):
  - G per-channel DMAs into padded scan tile xp. Per-channel slot layout
    (free axis, SEG=530): [zs][t0][t1][t2][t3]; t-seg = 9z|512 data|9z,
    zs = 530 zeros. Pads memset once, never rewritten.
  - ONE tensor_tensor_scan (fp32 state) = 9-tap W-box for every row.
  - H-box via a doubling tree where the partition shift of each level is
    materialized by a DMA pair (engines cannot read partition-offset APs;
    DMA can): main shift SH[0:128-s) <- src[s:128) plus chunk-boundary
    wrap SH[128-s:128, c) <- src[0:s, c+SEG). Then one full 2D
    tensor_tensor add. The zs segments accumulate the image-edge partial
    sums exactly as in the validated numpy mock (mock_v3).
  - Final: P = B3 + shift(+8)(B0); O = shift(-4)(P) via 2 DMAs into V.
  - G per-channel DMAs out of the value columns.
"""

import numpy as np
import ml_dtypes

import concourse.mybir as mybir
import concourse.tile as tile
from concourse import bacc, bass_utils

RADIUS = 4
H = W = 512
P = 128
NCHUNK = 4
N_CORES = 8
NCH = 32

SEG = 530
CH = 5 * SEG
G = 9
LTS = G * CH + SEG + 9   # tree/scan-out tile length (tail zs + slack)
LT = LTS + 9             # xp length

BF16 = mybir.dt.bfloat16
ADD = mybir.AluOpType.add


NSPLIT = 1  # col-chunks per main shift (1: split adds only dispatch cost)


def _shift_up(nc, dst, src, s):
    """dst[p] = src[p+s] rows-wise with chunk wrap: top s partitions read
    the next segment (c+SEG) of the bottom s partitions. The main shift is
    issued as NSPLIT independent col-chunk DMAs so they run concurrently."""
    step = (LTS + NSPLIT - 1) // NSPLIT
    for j in range(0, LTS, step):
        e = min(j + step, LTS)
        nc.sync.dma_start(dst[0 : P - s, j:e], src[s:P, j:e])
    nc.sync.dma_start(dst[P - s : P, 0 : LTS - SEG], src[0:s, SEG:LTS])


def _emit_pass(nc, tiles, x_ap, y_ap, nch):
    xp, xs, a, b = tiles
    for g0 in range(0, nch, G):
        gn = min(G, nch - g0)
        for i in range(gn):
            cb = i * CH + SEG
            nc.sync.dma_start(
                xp[:, cb : cb + 4 * SEG]
                .rearrange("p (t c) -> p t c", t=NCHUNK)[:, :, 9 : 9 + W],
                x_ap[g0 + i].rearrange("(t p) w -> p t w", p=P),
            )
        # W pass: one scan
        nc.vector.tensor_tensor_scan(
            xs[:, 0:LTS], xp[:, 9:LT], xp[:, 0:LTS], 0.0,
            ADD, mybir.AluOpType.subtract,
        )
        # H pass: doubling tree (shifts 1,2,4), then P9 = B4 + sh8(xs).
        # b accumulates in place; a is the shift scratch.
        _shift_up(nc, a, xs, 1)
        nc.vector.tensor_tensor(b[:, 0:LTS], xs[:, 0:LTS], a[:, 0:LTS], ADD)
        for s in (2, 4):
            _shift_up(nc, a, b, s)
            nc.vector.tensor_tensor(
                b[:, 0:LTS], b[:, 0:LTS], a[:, 0:LTS], ADD
            )
        _shift_up(nc, a, xs, 8)
        nc.vector.tensor_tensor(b[:, 0:LTS], b[:, 0:LTS], a[:, 0:LTS], ADD)
        # the -4 centering shift folds into the out-DMAs: output row
        # 128d+p reads P9[p-4] (chunk d; prev seg's top 4 rows for p<4)
        for i in range(gn):
            dview = y_ap[g0 + i].rearrange("(d p) w -> p d w", p=P)
            # vv segs: j=0 is the slot zs, j=1..4 are chunks 0..3
            vv = b[:, i * CH : i * CH + 5 * SEG].rearrange(
                "p (j c) -> p j c", j=5
            )
            nc.sync.dma_start(
                dview[4:P], vv[0 : P - 4, 1 : NCHUNK + 1, 4 : 4 + W]
            )
            nc.sync.dma_start(
                dview[0:4], vv[P - 4 : P, 0:NCHUNK, 4 : 4 + W]
            )


def _build(nch: int, chain: int, tiny_io: bool):
    nc = bacc.Bacc("TRN2", target_bir_lowering=False, debug=False)
    xshape = [1, H, W] if tiny_io else [nch, H, W]
    x = nc.dram_tensor("x", xshape, BF16, kind="ExternalInput").ap()
    y = nc.dram_tensor("y", xshape, BF16, kind="ExternalOutput").ap()

    with tile.TileContext(nc) as tc:
        with (
            tc.tile_pool(name="big", bufs=1) as pool,
            tc.tile_pool(name="dram", bufs=2, space="DRAM") as dram_pool,
        ):
            xp = pool.tile([P, LT], BF16, tag="xp")
            xs = pool.tile([P, LTS], BF16, tag="xs")
            a = pool.tile([P, LTS], BF16, tag="a")
            b = pool.tile([P, LTS], BF16, tag="b")
            nc.vector.memset(xp[:], 0.0)
            nc.vector.memset(xs[:], 0.0)
            nc.vector.memset(a[:], 0.0)
            nc.vector.memset(b[:], 0.0)
            tiles = (xp, xs, a, b)

            if tiny_io:
                cur = _Bcast(x)
                for it in range(chain):
                    dst = dram_pool.tile([nch, H, W], BF16, tag="scr")
                    _emit_pass(nc, tiles, cur, dst, nch)
                    cur = dst
                nc.sync.dma_start(y[0], cur[0])
            else:
                cur = x
                for it in range(chain):
                    last = it == chain - 1
                    dst = (
                        y if last
                        else dram_pool.tile([nch, H, W], BF16, tag="scr")
                    )
                    _emit_pass(nc, tiles, cur, dst, nch)
                    cur = dst

    nc.compile()
    return nc


class _Bcast:
    def __init__(self, ap):
        self._ap = ap

    def __getitem__(self, c):
        return self._ap[0]


def build_nc(nch: int = NCH, chain: int = 1):
    return _build(nch, chain, tiny_io=False)


def build_bench(k: int, nch: int = NCH):
    return _build(nch, k, tiny_io=True)


def kernel(image) -> np.ndarray:
    image = np.asarray(image)
    assert image.shape == (N_CORES, NCH, H, W), image.shape
    image_bf = np.ascontiguousarray(image).astype(ml_dtypes.bfloat16)
    nc = build_nc(NCH)
    in_maps = [{"x": image_bf[b]} for b in range(N_CORES)]
    res = bass_utils.run_bass_kernel_spmd(nc, in_maps, core_ids=list(range(N_CORES)))
    return np.stack([r["y"].astype(np.float32) for r in res.results], axis=0)


if __name__ == "__main__":
    img = np.random.rand(N_CORES, NCH, H, W).astype(np.float32)
    out = kernel(img)
    print(out.shape, out.dtype)
